# revision 1
# baseline (speedup 1.0000x reference)
"""Trainium2 Bass kernel for nn_AbstractAffine (DeepPoly-style backsubstitution).

Math
----
Reference scans L=16 layers over relational bound state (xl, xu, bl, bu):
    pl = max(xl,0); nl = min(xl,0); pu = max(xu,0); nu = min(xu,0)
    bl += pl@c_lo + nl@c_up ;  bu += pu@c_up + nu@c_lo
    xl  = pl@A_lo + nl@A_up ;  xu  = pu@A_up + nu@A_lo
Using max(x,0)=(x+|x|)/2, min(x,0)=(x-|x|)/2 with S=(A_lo+A_up)/2,
Dm=(A_lo-A_up)/2:
    xl' = xl@S + |xl|@Dm ;  xu' = xu@S - |xu|@Dm
(same form for the bias vectors and for the final input-bound reduction).

Mapping
-------
Output-neuron rows are sharded over 8 cores (128 rows each). Each core keeps
its state TRANSPOSED (contraction index j on partitions), packed per j-chunk
as a (128, 512) tile [xlT | xuT | |xl|T | -|xu|T]. Then every matmul uses an
A-matrix slice (natural HBM layout) as the stationary operand and the packed
state as the moving operand (free dim 256), so no on-chip transposes are ever
needed. A-matrices are fed in fp16 (host-cast; adds ~5e-4 rel err), state in
fp16, accumulation in fp32 PSUM, bias vectors in fp32.
"""

import numpy as np
from contextlib import ExitStack

import concourse.bass as bass
import concourse.tile as tile
from concourse import bacc, mybir
from concourse.bass_utils import run_bass_kernel_spmd

L = 16
D = 1024
NCORES = 8
R = D // NCORES          # rows per core
JC = D // 128            # j-chunks

A_DT = mybir.dt.float16
ST_DT = mybir.dt.float16
A_NP = np.float16
ST_NP = np.float16

_CACHE = {}


def _build():
    f32 = mybir.dt.float32
    nc = bacc.Bacc(None, target_bir_lowering=False)
    s_dram = nc.dram_tensor("s_mats", [L, 128, JC * 1024], A_DT, kind="ExternalInput")
    d_dram = nc.dram_tensor("d_mats", [L, 128, JC * 1024], A_DT, kind="ExternalInput")
    scdc_dram = nc.dram_tensor("scdc", [128, JC * (2 * L + 2)], A_DT,
                               kind="ExternalInput")
    init_dram = nc.dram_tensor("init_t", [128, JC * 512], ST_DT, kind="ExternalInput")
    bias_dram = nc.dram_tensor("bias0", [1, 256], f32, kind="ExternalInput")
    out_dram = nc.dram_tensor("out", [1, 256], f32, kind="ExternalOutput")
    SDW = 2 * L + 2  # scdc columns per j-chunk

    MULT = mybir.AluOpType.mult
    MIN = mybir.AluOpType.min
    MAX = mybir.AluOpType.max
    ADD = mybir.AluOpType.add

    with tile.TileContext(nc) as tc:
        with ExitStack() as ctx:
            apool = ctx.enter_context(tc.tile_pool(name="amat", bufs=12))
            spool = ctx.enter_context(tc.tile_pool(name="state", bufs=16))
            cpool = ctx.enter_context(tc.tile_pool(name="consts", bufs=1))
            ppool = ctx.enter_context(tc.tile_pool(name="psum", bufs=1, space="PSUM"))

            # PE warm-up: dummy matmuls on an uninitialized tile into a trash
            # PSUM bank while the first DMAs are in flight — releases the HAM
            # clock throttle before the first real matmul arrives
            dummy = cpool.tile([128, 256], A_DT, tag="dummy")
            nc.vector.memset(dummy[:], 0.0)
            trash = ppool.tile([128, 256], f32, tag="ps7")
            for i in range(32):
                nc.tensor.matmul(trash[:], dummy[:, 0:128], dummy[:],
                                 start=(i == 0), stop=(i == 31))

            # const loads on the scalar engine's HWDGE path so the sync
            # queue's first triggers are already layer-0 data; t0 quarters
            # first (they gate the first matmuls), scdc/bias later
            t0_all = cpool.tile([128, JC * 512], ST_DT, tag="t0")
            for q in range(4):
                nc.scalar.dma_start(t0_all[:, q * 1024:(q + 1) * 1024],
                                    init_dram[:, q * 1024:(q + 1) * 1024])
            scdc_all = cpool.tile([128, JC * SDW], A_DT, tag="scdc")
            nc.scalar.dma_start(scdc_all[:], scdc_dram[:])
            bias = cpool.tile([1, 256], f32, tag="bias")
            nc.scalar.dma_start(bias[:], bias_dram[:])
            scdc = [scdc_all[:, jc * SDW:(jc + 1) * SDW] for jc in range(JC)]
            T = [t0_all[:, jc * 512:(jc + 1) * 512] for jc in range(JC)]

            # bias accumulator: ACC[p, :] += sc[p]*state[p, :] + dc[p]*abs[p, :]
            # summed over all layers on DVE (axpys run during the matmul
            # phase); one ones-vector fp32 matmul at the very end reduces
            # over partitions. Keeps all PE cycles for coefficient matmuls.
            acc = cpool.tile([128, 256], f32, tag="acc")
            nc.vector.memset(acc[:], 0.0)
            ones = cpool.tile([128, 1], f32, tag="ones")
            nc.vector.memset(ones[:], 1.0)

            def bias_round(col_s, col_d):
                for jc in range(JC):
                    nc.vector.scalar_tensor_tensor(
                        acc[:], T[jc][:, 0:256], scdc[jc][:, col_s:col_s + 1],
                        acc[:], MULT, ADD)
                    nc.vector.scalar_tensor_tensor(
                        acc[:], T[jc][:, 256:512], scdc[jc][:, col_d:col_d + 1],
                        acc[:], MULT, ADD)

            for l in range(L):
                # A-matrix loads: quarters for layer 0 (earliest possible
                # first matmul), halves (1 MB) after — coarse enough to
                # amortize the per-dma_start trigger cost
                nchunk = 8 if l == 0 else (4 if l <= 2 else 2)
                cw = JC * 1024 // nchunk
                chunks = {}
                for h in range(nchunk):
                    for mat, dram in (("s", s_dram), ("d", d_dram)):
                        t = apool.tile([128, cw], A_DT, tag=f"amat{nchunk}",
                                       name=f"a{mat}{l}_{h}",
                                       bufs={8: 16, 4: 16, 2: 8}[nchunk])
                        nc.sync.dma_start(t[:], dram[l, :, h * cw:(h + 1) * cw])
                        chunks[(mat, h)] = t
                jpc = JC // nchunk  # j-chunks per dma chunk

                def aslice(mat, jc):
                    t = chunks[(mat, jc // jpc)]
                    return t[:, (jc % jpc) * 1024:(jc % jpc) * 1024 + 1024]

                bias_round(2 * l, 2 * l + 1)

                ps = [ppool.tile([128, 256], f32, tag=f"ps{i}", name=f"ps{i}_{l}")
                      for i in range(8)]
                newT = [spool.tile([128, 512], ST_DT, tag="T", name=f"T{j}_{l}")
                        for j in range(JC)]
                # jc-outer for all but the last j-chunk: consumes DMA chunks
                # as they land, all 8 cc accumulation groups open in their
                # own PSUM banks
                for jc in range(JC - 1):
                    for cc in range(8):
                        off = cc * 128
                        nc.tensor.matmul(
                            ps[cc][:], aslice("s", jc)[:, off:off + 128],
                            T[jc][:, 0:256], start=(jc == 0), stop=False)
                        nc.tensor.matmul(
                            ps[cc][:], aslice("d", jc)[:, off:off + 128],
                            T[jc][:, 256:512], start=False, stop=False)
                # last j-chunk cc-outer: staggers group closes so PSUM->SBUF
                # copies overlap the remaining matmuls
                jc = JC - 1
                last = (l == L - 1)
                for cc in range(8):
                    off = cc * 128
                    nc.tensor.matmul(
                        ps[cc][:], aslice("s", jc)[:, off:off + 128],
                        T[jc][:, 0:256], start=False, stop=False)
                    nc.tensor.matmul(
                        ps[cc][:], aslice("d", jc)[:, off:off + 128],
                        T[jc][:, 256:512], start=False, stop=True)
                    nt = newT[cc]
                    if last:
                        nc.vector.tensor_copy(nt[:, 0:256], ps[cc][:])
                    else:
                        nc.scalar.copy(nt[:, 0:256], ps[cc][:])
                    nc.scalar.activation(
                        nt[:, 256:384], nt[:, 0:128],
                        mybir.ActivationFunctionType.Abs)
                    nc.vector.scalar_tensor_tensor(
                        nt[:, 384:512], nt[:, 128:256], -1.0, nt[:, 128:256],
                        MULT, MIN)
                T = newT

            # partition-reduce the accumulator (plain-fp32 matmul; its input
            # was complete by mid-layer-15, so this runs without a stall)
            pb1 = ppool.tile([1, 256], f32, tag="ps1")
            nc.tensor.matmul(pb1[:], ones[:], acc[:], start=True, stop=True)
            # final input-bound round on PE (16 small matmuls) — cheaper at
            # the kernel tail than a serial 16-op DVE axpy chain
            pb2 = ppool.tile([1, 512], f32, tag="ps2")
            for jc in range(JC):
                nc.tensor.matmul(
                    pb2[:, 0:256], scdc[jc][:, 2 * L:2 * L + 1],
                    T[jc][:, 0:256], start=(jc == 0), stop=(jc == JC - 1))
            for jc in range(JC):
                nc.tensor.matmul(
                    pb2[:, 256:512], scdc[jc][:, 2 * L + 1:2 * L + 2],
                    T[jc][:, 256:512], start=(jc == 0), stop=(jc == JC - 1))
            nc.vector.tensor_add(bias[:], bias[:], pb1[:])
            nc.vector.tensor_add(bias[:], bias[:], pb2[:, 0:256])
            nc.vector.tensor_add(bias[:], bias[:], pb2[:, 256:512])
            nc.sync.dma_start(out_dram[:], bias[:])
    nc.compile()
    return nc


def _prep_inputs(weights, biases, net_x_lowers, net_x_uppers,
                 net_b_lowers, net_b_uppers, input_lowers, input_uppers):
    W = np.ascontiguousarray(np.asarray(weights, dtype=np.float32))
    b = np.asarray(biases, dtype=np.float32).reshape(D)
    AL = np.asarray(net_x_lowers, dtype=np.float32)
    AU = np.asarray(net_x_uppers, dtype=np.float32)
    cL = np.asarray(net_b_lowers, dtype=np.float32).reshape(L, D)
    cU = np.asarray(net_b_uppers, dtype=np.float32).reshape(L, D)
    lo = np.asarray(input_lowers, dtype=np.float32).reshape(D)
    up = np.asarray(input_uppers, dtype=np.float32).reshape(D)

    S = 0.5 * (AL + AU)
    Dm = 0.5 * (AL - AU)
    # (L, 128, JC*1024): [l, p, jc*1024 + c] = S[l, jc*128 + p, c]
    s_mats = np.ascontiguousarray(
        S.reshape(L, JC, 128, D).transpose(0, 2, 1, 3).reshape(L, 128, JC * D)
    ).astype(A_NP)
    d_mats = np.ascontiguousarray(
        Dm.reshape(L, JC, 128, D).transpose(0, 2, 1, 3).reshape(L, 128, JC * D)
    ).astype(A_NP)

    sc = 0.5 * (cL + cU)
    dc = 0.5 * (cL - cU)
    s_in = 0.5 * (lo + up)
    d_in = 0.5 * (lo - up)
    sd = np.empty((JC, 128, 2 * L + 2), np.float32)
    sd[:, :, 0:2 * L:2] = sc.reshape(L, JC, 128).transpose(1, 2, 0)
    sd[:, :, 1:2 * L:2] = dc.reshape(L, JC, 128).transpose(1, 2, 0)
    sd[:, :, 2 * L] = s_in.reshape(JC, 128)
    sd[:, :, 2 * L + 1] = d_in.reshape(JC, 128)
    # dram layout (128, JC*SDW): [p, jc*SDW + col]
    scdc = np.ascontiguousarray(
        sd.transpose(1, 0, 2).reshape(128, JC * (2 * L + 2))).astype(A_NP)

    Wh = W.astype(ST_NP)  # x0T[j, r] = W[j, r]; round once, abs of rounded
    Wr = Wh.reshape(JC, 128, D)
    in_maps = []
    for k in range(NCORES):
        cols = Wr[:, :, k * R:(k + 1) * R]
        initT = np.empty((JC, 128, 512), ST_NP)
        initT[:, :, 0:128] = cols
        initT[:, :, 128:256] = cols
        initT[:, :, 256:384] = np.abs(cols)
        initT[:, :, 384:512] = -np.abs(cols)
        # dram layout (128, JC*512): [p, jc*512 + c]
        initT = np.ascontiguousarray(
            initT.transpose(1, 0, 2).reshape(128, JC * 512))
        b0 = np.empty((1, 256), np.float32)
        b0[0, 0:128] = b[k * R:(k + 1) * R]
        b0[0, 128:256] = b[k * R:(k + 1) * R]
        in_maps.append({
            "s_mats": s_mats,
            "d_mats": d_mats,
            "scdc": scdc,
            "init_t": np.ascontiguousarray(initT),
            "bias0": b0,
        })
    return in_maps


def _run(inputs, trace=False):
    if "nc" not in _CACHE:
        _CACHE["nc"] = _build()
    nc = _CACHE["nc"]
    in_maps = _prep_inputs(**inputs)
    try:
        res = run_bass_kernel_spmd(nc, in_maps, core_ids=list(range(NCORES)),
                                   trace=trace)
    except Exception:
        # transient NRT device errors have been observed; retry once
        res = run_bass_kernel_spmd(nc, in_maps, core_ids=list(range(NCORES)),
                                   trace=trace)
    lowers = np.empty((D, 1), np.float32)
    uppers = np.empty((D, 1), np.float32)
    for k in range(NCORES):
        arr = res.results[k]["out"]
        lowers[k * R:(k + 1) * R, 0] = arr[0, 0:128]
        uppers[k * R:(k + 1) * R, 0] = arr[0, 128:256]
    out = np.stack([lowers, uppers])
    return out, res


def kernel(**inputs):
    out, _ = _run(inputs, trace=False)
    return out

